# revision 78
# baseline (speedup 1.0000x reference)
"""GQA kernel for Trainium2, 8 NeuronCores.

Sharding: core c = b*4 + g handles batch b, kv-head g (4 query heads).
Host sums the 4 partial outputs per batch.

Per core (all matmuls bf16, f32 PSUM):
  Q_h^T = Wq_h @ x_q^T   [128 d, S]
  K^T   = Wk  @ x_k^T    [128 d, S]
  V     = [S, 128]  (V^T via matmul, then PE transpose)
  S^T   = K_tile @ Q^T -> [k, q] scores; exp on scalar engine into
          bf16 P; causal diagonal handled by column-narrowed scores/
          exp + one gpsimd multiply with a [zeros|triangle] mask that
          both applies the causal block and zeroes the stale left
          region of the pool tile (pool buffers memset once at start)
  o^T  += V[kt] @ P[kt] (PSUM)
  l     = all P tiles of a head-chunk pairwise tree-summed on DVE
          (bf16), then ONE ones-row matmul [1,SC].  (A per-tile
          [1,SC] l matmul costs a full 512-column PE pass - measured
          0.47 ns/col, NO per-pass overhead - so moving the k-tile
          reduction off the PE saves ~33us of streaming.)
  norm  = reciprocal_approx_fast(l) [1,SC] (DVE) -> bf16 cast
          (scalar) -> ones[1,128] matmul broadcasts across partitions
          into PSUM (241ns PE) -> scalar copy to SBUF -> DVE mul
  out   = onrm @ Wo -> bf16 partial [S, E]

Perf notes (477us -> 381us -> 330us -> 283us):
 - PE pass cost measured LINEAR in streamed columns (0.42-0.47ns/col
   at full clock): wins = fewer streamed columns + fewer gaps.
 - fp8 DoubleRow measured only ~1.9x per k-tile on HW (not the cost
   model's 4x) and e4m3's 3.6% rms noise busts the 2e-2 budget on
   every tensor, so everything stays bf16.
 - exp costs ~0.8ns/col + 154ns/op on the scalar engine vs 482ns of
   PE score+AV work per k-tile, so the attention inner loop is
   exp-gated.  Q projections are software-pipelined across heads:
   passes of the NEXT head's projection and per-ec groups of the
   previous chunk's deferred outproj stripes are sprinkled into the
   kt loop as PE filler (denser near the narrow diagonal tail).
 - The previous head's normalize is staged across the next head's
   loop (reciprocal+cast @kt1, broadcast @kt3, multiply @kt5).  A
   DRAM-bounce broadcast measured ~5us latency and its
   data-dependent DMA triggers stalled the in-order scalar queue
   mid-exp-stream; gpsimd partition_broadcast thrashes microcode
   libraries (as does mixing gpsimd op types at all).
 - PSUM banks are the scarcest resource (8): acc=4 (scores + the
   held pipelined-proj psum + outproj), ops=2 (o of two heads in
   flight), 1 bank alternates l (row 0) with the 1/l broadcast
   (their lifetimes interleave exactly), 1 bank for V transposes.
   acc=3 cost ~15us in pool-wait stalls.
 - Host pre-layouts x / weights so each DMA row is one contiguous
   2-16KB run (was 256B-1KB rows = 2048 descriptors per load at
   ~5ns each of DIRECT2D sequencer time).  Startup is device-DMA-BW
   bound (~11MB over two hwdge queues); pieces arrive in first-use
   order (wk+xk interleaved, then wv, xv, per-head wq, xq) and
   chunk-0 projections consume them piece-by-piece.  The gpsimd
   swdge queue is NOT usable for bulk loads (per-descriptor
   microcode starves the triangle multiplies behind it).
 - Dummy 1-row PE passes are woven into the DMA-paced chunk-0
   projections to keep the 0.65->2.4GHz clock ramp alive through
   the waits.
 - All PSUM->SBUF copies run on DVE except the chunk-boundary qT
   copy and the broadcast stage (scalar), where the DVE queue is the
   deeper one.
"""

import sys

import numpy as np

for _p in ("/opt/trn_rl_repo",):
    if _p not in sys.path:
        sys.path.insert(0, _p)

import ml_dtypes

import concourse.bass as bass
import concourse.mybir as mybir
from concourse import bacc
from concourse.bass_utils import run_bass_kernel_spmd
from concourse.masks import make_identity
from concourse.tile import TileContext

B, S, E = 2, 2048, 2048
H, HKV = 16, 4
D = E // H  # 128
G = H // HKV  # 4 query heads per kv head
GD = G * D  # 512
NCORES = B * HKV  # 8
SC = 512  # s/q chunk width (free dim of matmuls)
NSC = S // SC  # 4
NET = E // 128  # 16 e-tiles (contraction)
NKT = S // 128  # 16 k-tiles
SCALE = 1.0 / float(np.sqrt(D))

F32 = mybir.dt.float32
BF16 = mybir.dt.bfloat16
AF = mybir.ActivationFunctionType
NPBF = np.dtype(ml_dtypes.bfloat16)

XW = NSC * NET * SC  # per-partition elems of an x tensor
WQW = NET * GD
WKW = NET * D
WOW = G * E
NPEXP = 6  # pexp pool buffers (memset at startup; trz reads stale cols)


def build_nc():
    nc = bacc.Bacc()
    # host pre-layout: row p of xq is, for sc then t:  x[sc*SC:(sc+1)*SC,
    # t*128+p] -- so one (sc, t-range) piece is a single contiguous run
    # per partition (16KB/chunk): 128 DMA descriptors instead of 2048.
    xq = nc.declare_dram_parameter("xq", [128, XW], BF16, isOutput=False)
    xk = nc.declare_dram_parameter("xk", [128, XW], BF16, isOutput=False)
    xv = nc.declare_dram_parameter("xv", [128, XW], BF16, isOutput=False)
    wq = nc.declare_dram_parameter("wq", [128, WQW], BF16, isOutput=False)
    wk = nc.declare_dram_parameter("wk", [128, WKW], BF16, isOutput=False)
    wv = nc.declare_dram_parameter("wv", [128, WKW], BF16, isOutput=False)
    wo = nc.declare_dram_parameter("wo", [128, WOW], BF16, isOutput=False)
    msk = nc.declare_dram_parameter("msk", [128, 512], BF16, isOutput=False)
    out = nc.declare_dram_parameter("out", [S, E], BF16, isOutput=True)

    def rows(dram, width, off=0, rw=None):
        # [128, width] contiguous-per-partition read of the flat layout
        base = dram[:, :]
        return bass.AP(
            tensor=base.tensor,
            offset=off,
            ap=[[rw if rw is not None else width, 128], [1, width]],
        )

    with TileContext(nc) as tc:
        with (
            tc.tile_pool(name="singles", bufs=1) as singles,
            tc.tile_pool(name="xt", bufs=5) as xtp,
            tc.tile_pool(name="pexp", bufs=NPEXP) as pexp,
            tc.tile_pool(name="lt", bufs=5) as ltp,
            tc.tile_pool(name="vt", bufs=2) as vtp,
            tc.tile_pool(name="rl", bufs=2) as rlp,
            tc.tile_pool(name="rsb", bufs=2) as rsbp,
            tc.tile_pool(name="ob", bufs=2) as obp,
            tc.tile_pool(name="acc", bufs=4, space="PSUM") as acc,
            tc.tile_pool(name="ops", bufs=2, space="PSUM") as ops,
            # l (colsum out, row 0 only) and the 1/l broadcast alternate in
            # ONE bank: l(h) is read (reciprocal) before rb(h) is written,
            # and rb(h) is read (norm mul) before l(h+1) is written
            tc.tile_pool(name="lrb", bufs=1, space="PSUM") as lrb,
            tc.tile_pool(name="trp", bufs=1, space="PSUM") as trp,
        ):
            # ---- constants / weights resident in SBUF ----
            # wq is stored head-major so head 0's slice can land first
            wq_sb = singles.tile([128, G, NET, D], BF16)  # 16KB/p
            wk_sb = singles.tile([128, NET, D], BF16)  # 4KB/p
            wv_sb = singles.tile([128, NET, D], BF16)  # 4KB/p
            wo_sb = singles.tile([128, G, E], BF16)  # 16KB/p
            trz = singles.tile([128, 512], BF16)  # [zeros 384 | tri] [k,q]
            ident_f = singles.tile([128, 128], F32)
            ident = singles.tile([128, 128], BF16)
            ones1 = singles.tile([128, 1], BF16)
            ones_bc = singles.tile([1, 128], BF16)
            qT = singles.tile([128, G, S], BF16)  # 16KB/p
            kT = singles.tile([128, S], BF16)  # 4KB/p
            v_sb = singles.tile([128, NKT, D], BF16)  # 4KB/p
            onrm = singles.tile([128, G, S], BF16)  # 16KB/p

            make_identity(nc, ident_f)
            nc.scalar.activation(out=ident[:], in_=ident_f[:], func=AF.Copy)
            nc.vector.memset(ones1, 1.0)
            nc.vector.memset(ones_bc, 1.0)
            # warm the pexp pool: trz-mul reads stale left columns of the
            # pool buffers, which must be finite (not uninit-NaN)
            wt0 = None
            for _ in range(NPEXP):
                wt = pexp.tile([128, SC], BF16, tag="p")
                nc.vector.memset(wt, 0.0)
                if wt0 is None:
                    wt0 = wt

            # clock-keeper: dummy PE passes emitted into the DMA-paced
            # startup so the 0.65->2.4GHz ramp completes during the waits
            # instead of inflating the first ~40us of real passes
            warm_ps = lrb.tile([128, SC], F32, tag="lrb", name="warm_ps")

            def emit_warm(n):
                for _ in range(n):
                    nc.tensor.matmul(
                        warm_ps[0:1], lhsT=ones1[:], rhs=wt0[:],
                        start=True, stop=True,
                    )

            emit_warm(4)

            xts = {}

            def load_chunk(sc):
                # xk/xv on sync, xq on scalar: one hwdge queue cannot keep a
                # full 6MB chunk ahead of the early (short) chunks.  (Chunk
                # 1's xk on scalar measured WORSE despite arriving earlier
                # on paper.)
                for nm, dram, eng in (
                    ("xk", xk, nc.sync),
                    ("xv", xv, nc.sync),
                    ("xq", xq, nc.scalar),
                ):
                    t = xtp.tile([128, NET, SC], BF16, tag="xt")
                    eng.dma_start(
                        out=t[:],
                        in_=rows(dram, NET * SC, off=sc * NET * SC, rw=XW),
                    )
                    xts[(nm, sc)] = t

            # startup: first-needed tensors land first, in 2/4-e-tile
            # pieces, so the first K-proj matmul issues ~2us in.
            t_xk = xtp.tile([128, NET, SC], BF16, tag="xt")
            t_xv = xtp.tile([128, NET, SC], BF16, tag="xt")
            t_xq = xtp.tile([128, NET, SC], BF16, tag="xt")
            xts[("xk", 0)] = t_xk
            xts[("xv", 0)] = t_xv
            xts[("xq", 0)] = t_xq
            # startup streams: sync carries xk -> xq -> xv (x of the first
            # chunk, in first-use order with AV tolerating a late V);
            # scalar carries weights with wq head-by-head.  K and Q
            # projections consume their pieces as they land.
            for g2 in range(NET // 2):  # 8 pieces of 2 e-tiles each
                nc.scalar.dma_start(
                    out=wk_sb[:, 2 * g2 : 2 * g2 + 2, :],
                    in_=rows(wk, 2 * D, off=2 * g2 * D, rw=WKW),
                )
                nc.sync.dma_start(
                    out=t_xk[:, 2 * g2 : 2 * g2 + 2, :],
                    in_=rows(xk, 2 * SC, off=2 * g2 * SC, rw=XW),
                )
            nc.scalar.dma_start(out=wv_sb[:], in_=rows(wv, WKW, rw=WKW))
            for g4 in range(NET // 4):  # 4 pieces of 4 e-tiles
                nc.sync.dma_start(
                    out=t_xv[:, 4 * g4 : 4 * g4 + 4, :],
                    in_=rows(xv, 4 * SC, off=4 * g4 * SC, rw=XW),
                )
            for hh in range(G):
                nc.scalar.dma_start(
                    out=wq_sb[:, hh],
                    in_=rows(wq, NET * D, off=hh * NET * D, rw=WQW),
                )
            for g4 in range(NET // 4):
                nc.sync.dma_start(
                    out=t_xq[:, 4 * g4 : 4 * g4 + 4, :],
                    in_=rows(xq, 4 * SC, off=4 * g4 * SC, rw=XW),
                )
            nc.scalar.dma_start(out=trz[:], in_=msk[:, :])
            nc.scalar.dma_start(out=wo_sb[:], in_=rows(wo, WOW, rw=WOW))

            def proj(w_sb, x_t, out_ps, hslice, warm_every=0):
                for t in range(NET):
                    nc.tensor.matmul(
                        out_ps[:],
                        lhsT=w_sb[:, t, hslice],
                        rhs=x_t[:, t, :],
                        start=(t == 0),
                        stop=(t == NET - 1),
                    )
                    if warm_every and t % warm_every == warm_every - 1:
                        emit_warm(1)

            # software-pipelined Q projection of the NEXT head
            pend = {}

            def pend_start(src_sc, hh):
                pend.clear()
                pend.update(
                    ps=acc.tile([128, SC], F32, tag="acc", name="pend_ps"),
                    xt=xts[("xq", src_sc)],
                    hh=hh,
                    ssl=slice(src_sc * SC, (src_sc + 1) * SC),
                    t=0,
                )

            def pend_pass():
                t = pend["t"]
                nc.tensor.matmul(
                    pend["ps"][:],
                    lhsT=wq_sb[:, pend["hh"], t, :],
                    rhs=pend["xt"][:, t, :],
                    start=(t == 0),
                    stop=(t == NET - 1),
                )
                pend["t"] = t + 1

            def pend_finish(eng=None):
                # at chunk boundaries the DVE queue is ~1.5us deep (tree
                # merges + norm + ob casts), so the h0 qT copy runs on the
                # scalar engine, which has drained its exps by then
                while pend["t"] < NET:
                    pend_pass()
                if eng is nc.scalar:
                    nc.scalar.activation(
                        out=qT[:, pend["hh"], pend["ssl"]],
                        in_=pend["ps"][:],
                        func=AF.Copy,
                    )
                else:
                    nc.vector.tensor_copy(
                        out=qT[:, pend["hh"], pend["ssl"]], in_=pend["ps"][:]
                    )
                pend.clear()

            # outproj stripes are emitted as per-ec atoms so they can be
            # sprinkled into an attention loop as PE filler; ostr is a
            # queue so a head can carry more than one deferred stripe
            ostr = []

            def ostripe_begin(c, sti):
                st = c * (SC // 128) + sti
                ostr.append(
                    dict(
                        stl=slice(st * 128, (st + 1) * 128),
                        last=st == S // 128 - 1,
                        ob=obp.tile([128, E], BF16, tag="ob", name="ob"),
                        ec=0,
                    )
                )

            def ostripe_ec():
                s = ostr[0]
                ec, stl, ob = s["ec"], s["stl"], s["ob"]
                esl = slice(ec * SC, (ec + 1) * SC)
                ps = acc.tile([128, SC], F32, tag="acc")
                for h in range(G):
                    nc.tensor.matmul(
                        ps[:],
                        lhsT=onrm[:, h, stl],
                        rhs=wo_sb[:, h, esl],
                        start=(h == 0),
                        stop=(h == G - 1),
                    )
                nc.vector.tensor_copy(out=ob[:, esl], in_=ps[:])
                if s["last"]:
                    # final stripe: per-ec DMA so the tail write overlaps
                    # the remaining ec compute
                    nc.gpsimd.dma_start(out=out[stl, esl], in_=ob[:, esl])
                s["ec"] = ec + 1
                if s["ec"] == E // SC:
                    if not s["last"]:
                        nc.gpsimd.dma_start(out=out[stl, :], in_=ob[:])
                    ostr.pop(0)

            def emit_outproj_stripe(c, sti):
                ostripe_begin(c, sti)
                while ostr:
                    ostripe_ec()

            norm_pend = {}

            def emit_norm_a1():
                # 1/l on [1,SC] (DVE) + bf16 cast (scalar)
                if "v" not in norm_pend:
                    return
                o_ps, l_ps, hh, ssl = norm_pend.pop("v")
                rl = rlp.tile([1, SC], F32, tag="rl")
                nc.vector.reciprocal_approx_fast(out=rl[:], in_=l_ps[:])
                rlb = rlp.tile([1, SC], BF16, tag="rlb")
                # cast on DVE: a scalar-engine op here would push the whole
                # in-order exp stream back by its duration every head
                nc.vector.tensor_copy(out=rlb[:], in_=rl[:])
                norm_pend["v1"] = (o_ps, rlb, hh, ssl)

            def emit_norm_a2():
                # a 1-row ones matmul broadcasts 1/l across partitions into
                # PSUM.  (A DRAM-bounce broadcast measured ~5us of latency
                # and its data-dependent DMA triggers stalled the in-order
                # scalar queue right in the exp stream; gpsimd
                # partition_broadcast thrashes pool microcode libraries.)
                if "v1" not in norm_pend:
                    return
                o_ps, rlb, hh, ssl = norm_pend.pop("v1")
                rb_ps = lrb.tile([128, SC], F32, tag="lrb", name="rb_ps")
                nc.tensor.matmul(
                    rb_ps[:], lhsT=ones_bc[:], rhs=rlb[:], start=True, stop=True
                )
                norm_pend["v2"] = (o_ps, rb_ps, hh, ssl)

            def emit_norm_b():
                # two PSUM inputs on one DVE op are rejected by the BIR
                # verifier: stage the broadcast through SBUF on the scalar
                # engine (it has slack), then multiply on DVE
                if "v2" not in norm_pend:
                    return
                o_ps, rb_ps, hh, ssl = norm_pend.pop("v2")
                rb_sb = rsbp.tile([128, SC], BF16, tag="rbsb")
                nc.vector.tensor_copy(out=rb_sb[:], in_=rb_ps[:])
                nc.vector.tensor_mul(onrm[:, hh, ssl], o_ps[:], rb_sb[:])

            for sc in range(NSC):
                ssl = slice(sc * SC, (sc + 1) * SC)
                if sc + 1 < NSC:
                    load_chunk(sc + 1)
                if pend:  # remainder of this chunk's h0 Q projection
                    pend_finish(eng=nc.scalar)
                # K projection (chunk 0 is DMA-piece-paced: keep the PE
                # clock ramp alive through the waits with dummy passes)
                ps = acc.tile([128, SC], F32, tag="acc")
                proj(wk_sb, xts[("xk", sc)], ps, slice(0, D),
                     warm_every=2 if sc == 0 else 0)
                nc.vector.tensor_copy(out=kT[:, ssl], in_=ps[:])
                # V projection + transpose to [s, d]
                ps = acc.tile([128, SC], F32, tag="acc")
                proj(wv_sb, xts[("xv", sc)], ps, slice(0, D),
                     warm_every=4 if sc == 0 else 0)
                # vt copy on DVE: the scalar queue backs up ~20us of exps
                # at chunk boundaries and would stall the transposes
                vt = vtp.tile([128, SC], BF16, tag="vt")
                nc.vector.tensor_copy(out=vt[:], in_=ps[:])

                def emit_vtrans(i):
                    tp = trp.tile([128, 128], BF16, tag="tr")
                    nc.tensor.transpose(
                        tp[:], vt[:, i * 128 : (i + 1) * 128], ident[:]
                    )
                    nc.vector.tensor_copy(out=v_sb[:, sc * 4 + i, :], in_=tp[:])

                for i in range(SC // 128):
                    emit_vtrans(i)

                nkt = (sc + 1) * (SC // 128)  # causal: k tiles 0..nkt-1
                for h in range(G):
                    if sc == 0 and h == 0:
                        ps = acc.tile([128, SC], F32, tag="acc")
                        for t in range(NET):
                            nc.tensor.matmul(
                                ps[:],
                                lhsT=wq_sb[:, 0, t, :],
                                rhs=xts[("xq", 0)][:, t, :],
                                start=(t == 0),
                                stop=(t == NET - 1),
                            )
                            if t % 2 == 1:
                                emit_warm(1)
                        nc.vector.tensor_copy(out=qT[:, 0, ssl], in_=ps[:])
                    elif pend:
                        pend_finish()

                    # deferred outproj stripe of the previous chunk (its
                    # norms all completed by the previous head): queued
                    # behind the first scores so the exp stream restarts
                    # before this 3.9us PE block runs
                    if h > 0 and sc > 0:
                        ostripe_begin(sc - 1, h - 1)

                    # queue up the next head's Q projection for sprinkling
                    if h + 1 < G:
                        pend_start(sc, h + 1)
                    elif sc + 1 < NSC:
                        pend_start(sc + 1, 0)

                    # last head: sprinkle the previous chunk's final stripe
                    # into this loop as extra PE filler (all its norms are
                    # complete)
                    if h == G - 1 and sc > 0:
                        ostripe_begin(sc - 1, G - 1)

                    o_ps = ops.tile([128, SC], F32, tag="o")
                    l_full = lrb.tile([128, SC], F32, tag="lrb", name="l_ps")
                    l_ps = l_full[0:1]
                    pps = [None] * nkt
                    # binary-counter tree of P-tile sums (DVE, bf16)
                    lstack = []

                    def tree_push(t):
                        # level-1 pair adds ride the gpsimd queue (in
                        # emission order after the trz multiplies, so AV
                        # never queues behind them); upper levels on DVE,
                        # keeping the DVE queue short for the critical qT
                        # copies
                        rank = 1
                        while lstack and lstack[-1][1] == rank:
                            prev, _ = lstack.pop()
                            nt = ltp.tile([128, SC], BF16, tag="lt")
                            nc.vector.tensor_add(nt[:], prev[:], t[:])
                            t, rank = nt, rank * 2
                        lstack.append((t, rank))

                    def emit_scores(kt):
                        pp = pexp.tile([128, SC], BF16, tag="p")
                        pps[kt] = pp
                        jj = kt - (nkt - 4)
                        w0 = max(jj, 0) * 128  # first unmasked column
                        s_ps = acc.tile([128, SC], F32, tag="acc")
                        nc.tensor.matmul(
                            s_ps[:, w0:SC],
                            lhsT=kT[:, kt * 128 : (kt + 1) * 128],
                            rhs=qT[:, h, sc * SC + w0 : (sc + 1) * SC],
                            start=True,
                            stop=True,
                        )
                        nc.scalar.activation(
                            out=pp[:, w0:SC],
                            in_=s_ps[:, w0:SC],
                            func=AF.Exp,
                            scale=SCALE,
                        )
                        if jj >= 0:
                            # gpsimd applies only the 128-wide triangle
                            # block (short exp->AV critical path); the
                            # stale left region is zeroed off-path on DVE
                            # for the tree read
                            nc.gpsimd.tensor_mul(
                                pp[:, w0 : w0 + 128],
                                pp[:, w0 : w0 + 128],
                                trz[:, 384:512],
                            )
                            if w0 > 0:
                                nc.vector.memset(pp[:, 0:w0], 0.0)
                        tree_push(pp)

                    def emit_av(kt):
                        # masked (zero) columns of diagonal tiles are
                        # skipped: kt==0 is always full width so start=True
                        # initializes the whole PSUM region.
                        pp = pps[kt]
                        w0 = max(kt - (nkt - 4), 0) * 128
                        nc.tensor.matmul(
                            o_ps[:, w0:SC],
                            lhsT=v_sb[:, kt, :],
                            rhs=pp[:, w0:SC],
                            start=(kt == 0),
                            stop=(kt == nkt - 1),
                        )

                    for kt in range(nkt):
                        emit_scores(kt)
                        if kt == 1:
                            # previous head's normalize: the cast is emitted
                            # after exp(kt0)/exp(kt1) so the scalar queue
                            # keeps feeding the exp stream first
                            emit_norm_a1()
                        if kt == 3:
                            emit_norm_a2()
                        if kt == 5:
                            emit_norm_b()
                        if (kt % 4 == 2 or kt >= nkt - 3) and pend and pend[
                            "t"
                        ] < NET:
                            pend_pass()
                            if kt >= nkt - 3 and pend and pend["t"] < NET:
                                pend_pass()
                        if (kt % 4 == 3 or kt == nkt - 2) and ostr:
                            ostripe_ec()
                        if kt >= 1:
                            emit_av(kt - 1)
                    emit_av(nkt - 1)
                    emit_norm_a2()  # no-ops unless nkt was too short
                    emit_norm_b()
                    while ostr:
                        ostripe_ec()

                    # l: merge remaining tree levels, one column-sum matmul
                    lsum = None
                    for t, _r in lstack:
                        if lsum is None:
                            lsum = t
                        else:
                            nt = ltp.tile([128, SC], BF16, tag="lt")
                            nc.vector.tensor_add(nt[:], lsum[:], t[:])
                            lsum = nt
                    nc.tensor.matmul(
                        l_ps[:], lhsT=ones1[:], rhs=lsum[:], start=True, stop=True
                    )
                    norm_pend["v"] = (o_ps, l_ps, h, ssl)

            # last head of last chunk: overlap its normalize chain with the
            # h<3 passes of the first final stripe (h3 deferred past the mul)
            emit_norm_a1()
            st = (NSC - 1) * (SC // 128)
            stl0 = slice(st * 128, (st + 1) * 128)
            ob0 = obp.tile([128, E], BF16, tag="ob", name="ob0")
            pss = []
            for ec in range(E // SC):
                esl = slice(ec * SC, (ec + 1) * SC)
                ps = acc.tile([128, SC], F32, tag="acc")
                pss.append(ps)
                for h in range(G - 1):
                    nc.tensor.matmul(
                        ps[:],
                        lhsT=onrm[:, h, stl0],
                        rhs=wo_sb[:, h, esl],
                        start=(h == 0),
                        stop=False,
                    )
                if ec == 0:
                    emit_norm_a2()
            emit_norm_b()
            for ec in range(E // SC):
                esl = slice(ec * SC, (ec + 1) * SC)
                nc.tensor.matmul(
                    pss[ec][:],
                    lhsT=onrm[:, G - 1, stl0],
                    rhs=wo_sb[:, G - 1, esl],
                    start=False,
                    stop=True,
                )
                nc.vector.tensor_copy(out=ob0[:, esl], in_=pss[ec][:])
                nc.gpsimd.dma_start(out=out[stl0, esl], in_=ob0[:, esl])
            for sti in range(1, SC // 128):
                emit_outproj_stripe(NSC - 1, sti)
    nc.compile()
    return nc


_NC_CACHE = None


def _get_nc():
    global _NC_CACHE
    if _NC_CACHE is None:
        _NC_CACHE = build_nc()
    return _NC_CACHE


def _prep_in_maps(query, key, value, attn_mask, Wq, Wk, Wv, Wo):
    query = np.asarray(query, dtype=np.float32)
    key = np.asarray(key, dtype=np.float32)
    value = np.asarray(value, dtype=np.float32)
    Wq = np.asarray(Wq, dtype=np.float32)
    Wk = np.asarray(Wk, dtype=np.float32)
    Wv = np.asarray(Wv, dtype=np.float32)
    Wo = np.asarray(Wo, dtype=np.float32)
    am = np.asarray(attn_mask)

    def xflat(x):
        # [S, E] -> [128, NSC*NET*SC]: row p = for sc, for t:
        # x[sc*SC:(sc+1)*SC, t*128+p]
        a = x.reshape(NSC, SC, NET, 128).transpose(3, 0, 2, 1)
        return np.ascontiguousarray(a.reshape(128, XW)).astype(NPBF)

    def wflat(w_t):  # w_t: [E, M] = W.T; -> [128, NET*M] row p = w_t[t*128+p]
        m = w_t.shape[1]
        a = w_t.reshape(NET, 128, m).transpose(1, 0, 2)
        return np.ascontiguousarray(a.reshape(128, NET * m)).astype(NPBF)

    def wqflat(w_t):  # [E, GD] -> [128, G*NET*D], head-major blocks
        a = w_t.reshape(NET, 128, G, D).transpose(1, 2, 0, 3)
        return np.ascontiguousarray(a.reshape(128, G * NET * D)).astype(NPBF)

    def woflat(w_g):  # w_g: [GD, E]; -> [128, G*E] row p = w_g[h*128+p]
        a = w_g.reshape(G, 128, E).transpose(1, 0, 2)
        return np.ascontiguousarray(a.reshape(128, G * E)).astype(NPBF)

    xqf = [xflat(query[b]) for b in range(B)]
    xkf = [xflat(key[b]) for b in range(B)]
    xvf = [xflat(value[b]) for b in range(B)]

    # [k, q] multiplicative triangle for the diagonal 128x128 blocks,
    # left-padded with 384 zero columns (stale-region zeroing slices)
    m0 = np.asarray(am[0, 0, :128, :128], dtype=np.float32)  # [q, k]
    msk2 = np.zeros((128, 512), np.float32)
    msk2[:, 384:512] = m0.T
    msk2 = np.ascontiguousarray(msk2).astype(NPBF)

    in_maps = []
    for b in range(B):
        for g in range(HKV):
            in_maps.append(
                {
                    "xq": xqf[b],
                    "xk": xkf[b],
                    "xv": xvf[b],
                    "wq": wqflat(
                        np.ascontiguousarray(Wq[g * GD : (g + 1) * GD, :].T)
                    ),
                    "wk": wflat(
                        np.ascontiguousarray(Wk[g * D : (g + 1) * D, :].T)
                    ),
                    "wv": wflat(
                        np.ascontiguousarray(Wv[g * D : (g + 1) * D, :].T)
                    ),
                    "wo": woflat(
                        np.ascontiguousarray(Wo[:, g * GD : (g + 1) * GD].T)
                    ),
                    "msk": msk2,
                }
            )
    return in_maps


def _run(inputs, trace=False, **kw):
    nc = _get_nc()
    in_maps = _prep_in_maps(**inputs)
    res = run_bass_kernel_spmd(
        nc, in_maps, list(range(NCORES)), trace=trace, **kw
    )
    outs = [np.asarray(r["out"]) for r in res.results]
    full = np.empty((B, S, E), dtype=np.float32)
    for b in range(B):
        acc = outs[b * HKV].astype(np.float32)
        for g in range(1, HKV):
            acc = acc + outs[b * HKV + g].astype(np.float32)
        full[b] = acc
    return full, res


def kernel(**inputs):
    full, _ = _run(inputs, trace=False)
    return full


# revision 79
# speedup vs baseline: 1.0235x; 1.0235x over previous
"""GQA kernel for Trainium2, 8 NeuronCores.

Sharding: core c = b*4 + g handles batch b, kv-head g (4 query heads).
Host sums the 4 partial outputs per batch.

Per core (all matmuls bf16, f32 PSUM):
  Q_h^T = Wq_h @ x_q^T   [128 d, S]
  K^T   = Wk  @ x_k^T    [128 d, S]
  V     = [S, 128]  (V^T via matmul, then PE transpose)
  S^T   = K_tile @ Q^T -> [k, q] scores; exp on scalar engine into
          bf16 P; causal diagonal handled by column-narrowed scores/
          exp + one gpsimd multiply with a [zeros|triangle] mask that
          both applies the causal block and zeroes the stale left
          region of the pool tile (pool buffers memset once at start)
  o^T  += V[kt] @ P[kt] (PSUM)
  l     = all P tiles of a head-chunk pairwise tree-summed on DVE
          (bf16), then ONE ones-row matmul [1,SC].  (A per-tile
          [1,SC] l matmul costs a full 512-column PE pass - measured
          0.47 ns/col, NO per-pass overhead - so moving the k-tile
          reduction off the PE saves ~33us of streaming.)
  norm  = reciprocal_approx_fast(l) [1,SC] (DVE) -> bf16 cast
          (scalar) -> ones[1,128] matmul broadcasts across partitions
          into PSUM (241ns PE) -> scalar copy to SBUF -> DVE mul
  out   = onrm @ Wo -> bf16 partial [S, E]

Perf notes (477us -> 381us -> 330us -> 283us):
 - PE pass cost measured LINEAR in streamed columns (0.42-0.47ns/col
   at full clock): wins = fewer streamed columns + fewer gaps.
 - fp8 DoubleRow measured only ~1.9x per k-tile on HW (not the cost
   model's 4x) and e4m3's 3.6% rms noise busts the 2e-2 budget on
   every tensor, so everything stays bf16.
 - exp costs ~0.8ns/col + 154ns/op on the scalar engine vs 482ns of
   PE score+AV work per k-tile, so the attention inner loop is
   exp-gated.  Q projections are software-pipelined across heads:
   passes of the NEXT head's projection and per-ec groups of the
   previous chunk's deferred outproj stripes are sprinkled into the
   kt loop as PE filler (denser near the narrow diagonal tail).
 - The previous head's normalize is staged across the next head's
   loop (reciprocal+cast @kt1, broadcast @kt3, multiply @kt5).  A
   DRAM-bounce broadcast measured ~5us latency and its
   data-dependent DMA triggers stalled the in-order scalar queue
   mid-exp-stream; gpsimd partition_broadcast thrashes microcode
   libraries (as does mixing gpsimd op types at all).
 - PSUM banks are the scarcest resource (8): acc=4 (scores + the
   held pipelined-proj psum + outproj), ops=2 (o of two heads in
   flight), 1 bank alternates l (row 0) with the 1/l broadcast
   (their lifetimes interleave exactly), 1 bank for V transposes.
   acc=3 cost ~15us in pool-wait stalls.
 - Host pre-layouts x / weights so each DMA row is one contiguous
   2-16KB run (was 256B-1KB rows = 2048 descriptors per load at
   ~5ns each of DIRECT2D sequencer time).  Startup is device-DMA-BW
   bound (~11MB over two hwdge queues); pieces arrive in first-use
   order (wk+xk interleaved, then wv, xv, per-head wq, xq) and
   chunk-0 projections consume them piece-by-piece.  The gpsimd
   swdge queue is NOT usable for bulk loads (per-descriptor
   microcode starves the triangle multiplies behind it).
 - Dummy 1-row PE passes are woven into the DMA-paced chunk-0
   projections to keep the 0.65->2.4GHz clock ramp alive through
   the waits.
 - All PSUM->SBUF copies run on DVE except the chunk-boundary qT
   copy and the broadcast stage (scalar), where the DVE queue is the
   deeper one.
"""

import sys

import numpy as np

for _p in ("/opt/trn_rl_repo",):
    if _p not in sys.path:
        sys.path.insert(0, _p)

import ml_dtypes

import concourse.bass as bass
import concourse.mybir as mybir
from concourse import bacc
from concourse.bass_utils import run_bass_kernel_spmd
from concourse.masks import make_identity
from concourse.tile import TileContext

B, S, E = 2, 2048, 2048
H, HKV = 16, 4
D = E // H  # 128
G = H // HKV  # 4 query heads per kv head
GD = G * D  # 512
NCORES = B * HKV  # 8
SC = 512  # s/q chunk width (free dim of matmuls)
NSC = S // SC  # 4
NET = E // 128  # 16 e-tiles (contraction)
NKT = S // 128  # 16 k-tiles
SCALE = 1.0 / float(np.sqrt(D))

F32 = mybir.dt.float32
BF16 = mybir.dt.bfloat16
AF = mybir.ActivationFunctionType
NPBF = np.dtype(ml_dtypes.bfloat16)

XW = NSC * NET * SC  # per-partition elems of an x tensor
WQW = NET * GD
WKW = NET * D
WOW = G * E
NPEXP = 6  # pexp pool buffers (memset at startup; trz reads stale cols)


def build_nc():
    nc = bacc.Bacc()
    # host pre-layout: row p of xq is, for sc then t:  x[sc*SC:(sc+1)*SC,
    # t*128+p] -- so one (sc, t-range) piece is a single contiguous run
    # per partition (16KB/chunk): 128 DMA descriptors instead of 2048.
    xq = nc.declare_dram_parameter("xq", [128, XW], BF16, isOutput=False)
    xk = nc.declare_dram_parameter("xk", [128, XW], BF16, isOutput=False)
    xv = nc.declare_dram_parameter("xv", [128, XW], BF16, isOutput=False)
    wq = nc.declare_dram_parameter("wq", [128, WQW], BF16, isOutput=False)
    wk = nc.declare_dram_parameter("wk", [128, WKW], BF16, isOutput=False)
    wv = nc.declare_dram_parameter("wv", [128, WKW], BF16, isOutput=False)
    wo = nc.declare_dram_parameter("wo", [128, WOW], BF16, isOutput=False)
    msk = nc.declare_dram_parameter("msk", [128, 512], BF16, isOutput=False)
    out = nc.declare_dram_parameter("out", [S, E], BF16, isOutput=True)

    def rows(dram, width, off=0, rw=None):
        # [128, width] contiguous-per-partition read of the flat layout
        base = dram[:, :]
        return bass.AP(
            tensor=base.tensor,
            offset=off,
            ap=[[rw if rw is not None else width, 128], [1, width]],
        )

    with TileContext(nc) as tc:
        with (
            tc.tile_pool(name="singles", bufs=1) as singles,
            tc.tile_pool(name="xt", bufs=5) as xtp,
            tc.tile_pool(name="pexp", bufs=NPEXP) as pexp,
            tc.tile_pool(name="lt", bufs=5) as ltp,
            tc.tile_pool(name="vt", bufs=2) as vtp,
            tc.tile_pool(name="rl", bufs=2) as rlp,
            tc.tile_pool(name="rsb", bufs=2) as rsbp,
            tc.tile_pool(name="ob", bufs=2) as obp,
            tc.tile_pool(name="acc", bufs=4, space="PSUM") as acc,
            tc.tile_pool(name="ops", bufs=2, space="PSUM") as ops,
            # l (colsum out, row 0 only) and the 1/l broadcast alternate in
            # ONE bank: l(h) is read (reciprocal) before rb(h) is written,
            # and rb(h) is read (norm mul) before l(h+1) is written
            tc.tile_pool(name="lrb", bufs=1, space="PSUM") as lrb,
            tc.tile_pool(name="trp", bufs=1, space="PSUM") as trp,
        ):
            # ---- constants / weights resident in SBUF ----
            # wq is stored head-major so head 0's slice can land first
            wq_sb = singles.tile([128, G, NET, D], BF16)  # 16KB/p
            wk_sb = singles.tile([128, NET, D], BF16)  # 4KB/p
            wv_sb = singles.tile([128, NET, D], BF16)  # 4KB/p
            wo_sb = singles.tile([128, G, E], BF16)  # 16KB/p
            trz = singles.tile([128, 512], BF16)  # [zeros 384 | tri] [k,q]
            ident_f = singles.tile([128, 128], F32)
            ident = singles.tile([128, 128], BF16)
            ones1 = singles.tile([128, 1], BF16)
            ones_bc = singles.tile([1, 128], BF16)
            qT = singles.tile([128, G, S], BF16)  # 16KB/p
            kT = singles.tile([128, S], BF16)  # 4KB/p
            v_sb = singles.tile([128, NKT, D], BF16)  # 4KB/p
            onrm = singles.tile([128, G, S], BF16)  # 16KB/p

            make_identity(nc, ident_f)
            nc.scalar.activation(out=ident[:], in_=ident_f[:], func=AF.Copy)
            nc.vector.memset(ones1, 1.0)
            nc.vector.memset(ones_bc, 1.0)
            # warm the pexp pool: trz-mul reads stale left columns of the
            # pool buffers, which must be finite (not uninit-NaN)
            wt0 = None
            for _ in range(NPEXP):
                wt = pexp.tile([128, SC], BF16, tag="p")
                nc.vector.memset(wt, 0.0)
                if wt0 is None:
                    wt0 = wt

            # clock-keeper: dummy PE passes emitted into the DMA-paced
            # startup so the 0.65->2.4GHz ramp completes during the waits
            # instead of inflating the first ~40us of real passes
            warm_ps = lrb.tile([128, SC], F32, tag="lrb", name="warm_ps")

            def emit_warm(n):
                for _ in range(n):
                    nc.tensor.matmul(
                        warm_ps[0:1], lhsT=ones1[:], rhs=wt0[:],
                        start=True, stop=True,
                    )

            emit_warm(4)

            xts = {}

            def load_chunk(sc):
                # xk/xv on sync, xq on scalar: one hwdge queue cannot keep a
                # full 6MB chunk ahead of the early (short) chunks.  (Chunk
                # 1's xk on scalar measured WORSE despite arriving earlier
                # on paper.)
                for nm, dram, eng in (
                    ("xk", xk, nc.sync),
                    ("xv", xv, nc.sync),
                    ("xq", xq, nc.scalar),
                ):
                    t = xtp.tile([128, NET, SC], BF16, tag="xt")
                    eng.dma_start(
                        out=t[:],
                        in_=rows(dram, NET * SC, off=sc * NET * SC, rw=XW),
                    )
                    xts[(nm, sc)] = t

            # startup: first-needed tensors land first, in 2/4-e-tile
            # pieces, so the first K-proj matmul issues ~2us in.
            t_xk = xtp.tile([128, NET, SC], BF16, tag="xt")
            t_xv = xtp.tile([128, NET, SC], BF16, tag="xt")
            t_xq = xtp.tile([128, NET, SC], BF16, tag="xt")
            xts[("xk", 0)] = t_xk
            xts[("xv", 0)] = t_xv
            xts[("xq", 0)] = t_xq
            # startup streams: sync carries xk -> xq -> xv (x of the first
            # chunk, in first-use order with AV tolerating a late V);
            # scalar carries weights with wq head-by-head.  K and Q
            # projections consume their pieces as they land.
            for g2 in range(NET // 2):  # 8 pieces of 2 e-tiles each
                nc.scalar.dma_start(
                    out=wk_sb[:, 2 * g2 : 2 * g2 + 2, :],
                    in_=rows(wk, 2 * D, off=2 * g2 * D, rw=WKW),
                )
                nc.sync.dma_start(
                    out=t_xk[:, 2 * g2 : 2 * g2 + 2, :],
                    in_=rows(xk, 2 * SC, off=2 * g2 * SC, rw=XW),
                )
            nc.scalar.dma_start(out=wv_sb[:], in_=rows(wv, WKW, rw=WKW))
            for g4 in range(NET // 4):  # 4 pieces of 4 e-tiles
                nc.sync.dma_start(
                    out=t_xv[:, 4 * g4 : 4 * g4 + 4, :],
                    in_=rows(xv, 4 * SC, off=4 * g4 * SC, rw=XW),
                )
            for hh in range(G):
                nc.scalar.dma_start(
                    out=wq_sb[:, hh],
                    in_=rows(wq, NET * D, off=hh * NET * D, rw=WQW),
                )
            for g4 in range(NET // 4):
                nc.sync.dma_start(
                    out=t_xq[:, 4 * g4 : 4 * g4 + 4, :],
                    in_=rows(xq, 4 * SC, off=4 * g4 * SC, rw=XW),
                )
            nc.scalar.dma_start(out=trz[:], in_=msk[:, :])
            nc.scalar.dma_start(out=wo_sb[:], in_=rows(wo, WOW, rw=WOW))

            def proj(w_sb, x_t, out_ps, hslice, warm_every=0):
                for t in range(NET):
                    nc.tensor.matmul(
                        out_ps[:],
                        lhsT=w_sb[:, t, hslice],
                        rhs=x_t[:, t, :],
                        start=(t == 0),
                        stop=(t == NET - 1),
                    )
                    if warm_every and t % warm_every == warm_every - 1:
                        emit_warm(1)

            # software-pipelined Q projection of the NEXT head
            pend = {}

            def pend_start(src_sc, hh):
                pend.clear()
                pend.update(
                    ps=acc.tile([128, SC], F32, tag="acc", name="pend_ps"),
                    xt=xts[("xq", src_sc)],
                    hh=hh,
                    ssl=slice(src_sc * SC, (src_sc + 1) * SC),
                    t=0,
                )

            def pend_pass():
                t = pend["t"]
                nc.tensor.matmul(
                    pend["ps"][:],
                    lhsT=wq_sb[:, pend["hh"], t, :],
                    rhs=pend["xt"][:, t, :],
                    start=(t == 0),
                    stop=(t == NET - 1),
                )
                pend["t"] = t + 1

            def pend_finish(eng=None):
                # at chunk boundaries the DVE queue is ~1.5us deep (tree
                # merges + norm + ob casts), so the h0 qT copy runs on the
                # scalar engine, which has drained its exps by then
                while pend["t"] < NET:
                    pend_pass()
                if eng is nc.scalar:
                    nc.scalar.activation(
                        out=qT[:, pend["hh"], pend["ssl"]],
                        in_=pend["ps"][:],
                        func=AF.Copy,
                    )
                else:
                    nc.vector.tensor_copy(
                        out=qT[:, pend["hh"], pend["ssl"]], in_=pend["ps"][:]
                    )
                pend.clear()

            # outproj stripes are emitted as per-ec atoms so they can be
            # sprinkled into an attention loop as PE filler; ostr is a
            # queue so a head can carry more than one deferred stripe
            ostr = []

            def ostripe_begin(c, sti):
                st = c * (SC // 128) + sti
                ostr.append(
                    dict(
                        stl=slice(st * 128, (st + 1) * 128),
                        last=st == S // 128 - 1,
                        ob=obp.tile([128, E], BF16, tag="ob", name="ob"),
                        ec=0,
                    )
                )

            def ostripe_ec():
                s = ostr[0]
                ec, stl, ob = s["ec"], s["stl"], s["ob"]
                esl = slice(ec * SC, (ec + 1) * SC)
                ps = acc.tile([128, SC], F32, tag="acc")
                for h in range(G):
                    nc.tensor.matmul(
                        ps[:],
                        lhsT=onrm[:, h, stl],
                        rhs=wo_sb[:, h, esl],
                        start=(h == 0),
                        stop=(h == G - 1),
                    )
                nc.vector.tensor_copy(out=ob[:, esl], in_=ps[:])
                if s["last"]:
                    # final stripe: per-ec DMA so the tail write overlaps
                    # the remaining ec compute
                    nc.gpsimd.dma_start(out=out[stl, esl], in_=ob[:, esl])
                s["ec"] = ec + 1
                if s["ec"] == E // SC:
                    if not s["last"]:
                        nc.gpsimd.dma_start(out=out[stl, :], in_=ob[:])
                    ostr.pop(0)

            def emit_outproj_stripe(c, sti):
                ostripe_begin(c, sti)
                while ostr:
                    ostripe_ec()

            norm_pend = {}

            def emit_norm_a1():
                # 1/l on [1,SC] (DVE) + bf16 cast (scalar)
                if "v" not in norm_pend:
                    return
                o_ps, l_ps, hh, ssl = norm_pend.pop("v")
                rl = rlp.tile([1, SC], F32, tag="rl")
                nc.vector.reciprocal_approx_fast(out=rl[:], in_=l_ps[:])
                rlb = rlp.tile([1, SC], BF16, tag="rlb")
                # cast on DVE: a scalar-engine op here would push the whole
                # in-order exp stream back by its duration every head
                nc.vector.tensor_copy(out=rlb[:], in_=rl[:])
                norm_pend["v1"] = (o_ps, rlb, hh, ssl)

            def emit_norm_a2():
                # a 1-row ones matmul broadcasts 1/l across partitions into
                # PSUM.  (A DRAM-bounce broadcast measured ~5us of latency
                # and its data-dependent DMA triggers stalled the in-order
                # scalar queue right in the exp stream; gpsimd
                # partition_broadcast thrashes pool microcode libraries.)
                if "v1" not in norm_pend:
                    return
                o_ps, rlb, hh, ssl = norm_pend.pop("v1")
                rb_ps = lrb.tile([128, SC], F32, tag="lrb", name="rb_ps")
                nc.tensor.matmul(
                    rb_ps[:], lhsT=ones_bc[:], rhs=rlb[:], start=True, stop=True
                )
                norm_pend["v2"] = (o_ps, rb_ps, hh, ssl)

            def emit_norm_b():
                # two PSUM inputs on one DVE op are rejected by the BIR
                # verifier: stage the broadcast through SBUF on the scalar
                # engine (it has slack), then multiply on DVE
                if "v2" not in norm_pend:
                    return
                o_ps, rb_ps, hh, ssl = norm_pend.pop("v2")
                rb_sb = rsbp.tile([128, SC], BF16, tag="rbsb")
                nc.vector.tensor_copy(out=rb_sb[:], in_=rb_ps[:])
                nc.vector.tensor_mul(onrm[:, hh, ssl], o_ps[:], rb_sb[:])

            for sc in range(NSC):
                ssl = slice(sc * SC, (sc + 1) * SC)
                if sc + 1 < NSC:
                    load_chunk(sc + 1)
                if pend:  # remainder of this chunk's h0 Q projection
                    pend_finish(eng=nc.scalar)
                # K projection (chunk 0 is DMA-piece-paced: keep the PE
                # clock ramp alive through the waits with dummy passes)
                ps = acc.tile([128, SC], F32, tag="acc")
                proj(wk_sb, xts[("xk", sc)], ps, slice(0, D),
                     warm_every=2 if sc == 0 else 0)
                nc.vector.tensor_copy(out=kT[:, ssl], in_=ps[:])
                # V projection + transpose to [s, d]
                ps = acc.tile([128, SC], F32, tag="acc")
                proj(wv_sb, xts[("xv", sc)], ps, slice(0, D),
                     warm_every=4 if sc == 0 else 0)
                # vt copy on DVE: the scalar queue backs up ~20us of exps
                # at chunk boundaries and would stall the transposes
                vt = vtp.tile([128, SC], BF16, tag="vt")
                nc.vector.tensor_copy(out=vt[:], in_=ps[:])

                def emit_vtrans(i):
                    tp = trp.tile([128, 128], BF16, tag="tr")
                    nc.tensor.transpose(
                        tp[:], vt[:, i * 128 : (i + 1) * 128], ident[:]
                    )
                    nc.vector.tensor_copy(out=v_sb[:, sc * 4 + i, :], in_=tp[:])

                for i in range(SC // 128):
                    emit_vtrans(i)

                nkt = (sc + 1) * (SC // 128)  # causal: k tiles 0..nkt-1
                for h in range(G):
                    if sc == 0 and h == 0:
                        ps = acc.tile([128, SC], F32, tag="acc")
                        for t in range(NET):
                            nc.tensor.matmul(
                                ps[:],
                                lhsT=wq_sb[:, 0, t, :],
                                rhs=xts[("xq", 0)][:, t, :],
                                start=(t == 0),
                                stop=(t == NET - 1),
                            )
                            if t % 2 == 1:
                                emit_warm(1)
                        nc.vector.tensor_copy(out=qT[:, 0, ssl], in_=ps[:])
                    elif pend:
                        pend_finish()

                    # deferred outproj stripe of the previous chunk (its
                    # norms all completed by the previous head): queued
                    # behind the first scores so the exp stream restarts
                    # before this 3.9us PE block runs
                    if h > 0 and sc > 0:
                        ostripe_begin(sc - 1, h - 1)

                    # queue up the next head's Q projection for sprinkling
                    if h + 1 < G:
                        pend_start(sc, h + 1)
                    elif sc + 1 < NSC:
                        pend_start(sc + 1, 0)

                    # last head: sprinkle the previous chunk's final stripe
                    # into this loop as extra PE filler (all its norms are
                    # complete)
                    if h == G - 1 and sc > 0:
                        ostripe_begin(sc - 1, G - 1)

                    o_ps = ops.tile([128, SC], F32, tag="o")
                    l_full = lrb.tile([128, SC], F32, tag="lrb", name="l_ps")
                    l_ps = l_full[0:1]
                    pps = [None] * nkt
                    # binary-counter tree of P-tile sums (DVE, bf16)
                    lstack = []

                    def tree_push(t):
                        # level-1 pair adds ride the gpsimd queue (in
                        # emission order after the trz multiplies, so AV
                        # never queues behind them); upper levels on DVE,
                        # keeping the DVE queue short for the critical qT
                        # copies
                        rank = 1
                        while lstack and lstack[-1][1] == rank:
                            prev, _ = lstack.pop()
                            nt = ltp.tile([128, SC], BF16, tag="lt")
                            nc.vector.tensor_add(nt[:], prev[:], t[:])
                            t, rank = nt, rank * 2
                        lstack.append((t, rank))

                    def emit_scores(kt):
                        pp = pexp.tile([128, SC], BF16, tag="p")
                        pps[kt] = pp
                        jj = kt - (nkt - 4)
                        w0 = max(jj, 0) * 128  # first unmasked column
                        s_ps = acc.tile([128, SC], F32, tag="acc")
                        nc.tensor.matmul(
                            s_ps[:, w0:SC],
                            lhsT=kT[:, kt * 128 : (kt + 1) * 128],
                            rhs=qT[:, h, sc * SC + w0 : (sc + 1) * SC],
                            start=True,
                            stop=True,
                        )
                        nc.scalar.activation(
                            out=pp[:, w0:SC],
                            in_=s_ps[:, w0:SC],
                            func=AF.Exp,
                            scale=SCALE,
                        )
                        if jj >= 0:
                            # gpsimd applies only the 128-wide triangle
                            # block (short exp->AV critical path); the
                            # stale left region is zeroed off-path on DVE
                            # for the tree read
                            nc.gpsimd.tensor_mul(
                                pp[:, w0 : w0 + 128],
                                pp[:, w0 : w0 + 128],
                                trz[:, 384:512],
                            )
                            if w0 > 0:
                                nc.vector.memset(pp[:, 0:w0], 0.0)
                        tree_push(pp)

                    def emit_av(kt):
                        # masked (zero) columns of diagonal tiles are
                        # skipped: kt==0 is always full width so start=True
                        # initializes the whole PSUM region.
                        pp = pps[kt]
                        w0 = max(kt - (nkt - 4), 0) * 128
                        nc.tensor.matmul(
                            o_ps[:, w0:SC],
                            lhsT=v_sb[:, kt, :],
                            rhs=pp[:, w0:SC],
                            start=(kt == 0),
                            stop=(kt == nkt - 1),
                        )

                    for kt in range(nkt):
                        emit_scores(kt)
                        if kt == 1:
                            # previous head's normalize: the cast is emitted
                            # after exp(kt0)/exp(kt1) so the scalar queue
                            # keeps feeding the exp stream first
                            emit_norm_a1()
                        if kt == 3:
                            emit_norm_a2()
                        if kt == 5:
                            emit_norm_b()
                        if (kt % 2 == 0 or kt >= nkt - 3) and pend and pend[
                            "t"
                        ] < NET:
                            pend_pass()
                            if kt >= nkt - 3 and pend and pend["t"] < NET:
                                pend_pass()
                        if (kt % 4 == 3 or kt == nkt - 2) and ostr:
                            ostripe_ec()
                        if kt >= 1:
                            emit_av(kt - 1)
                    emit_av(nkt - 1)
                    emit_norm_a2()  # no-ops unless nkt was too short
                    emit_norm_b()
                    while ostr:
                        ostripe_ec()

                    # l: merge remaining tree levels, one column-sum matmul
                    lsum = None
                    for t, _r in lstack:
                        if lsum is None:
                            lsum = t
                        else:
                            nt = ltp.tile([128, SC], BF16, tag="lt")
                            nc.vector.tensor_add(nt[:], lsum[:], t[:])
                            lsum = nt
                    nc.tensor.matmul(
                        l_ps[:], lhsT=ones1[:], rhs=lsum[:], start=True, stop=True
                    )
                    norm_pend["v"] = (o_ps, l_ps, h, ssl)

            # last head of last chunk: overlap its normalize chain with the
            # h<3 passes of the first final stripe (h3 deferred past the mul)
            emit_norm_a1()
            st = (NSC - 1) * (SC // 128)
            stl0 = slice(st * 128, (st + 1) * 128)
            ob0 = obp.tile([128, E], BF16, tag="ob", name="ob0")
            pss = []
            for ec in range(E // SC):
                esl = slice(ec * SC, (ec + 1) * SC)
                ps = acc.tile([128, SC], F32, tag="acc")
                pss.append(ps)
                for h in range(G - 1):
                    nc.tensor.matmul(
                        ps[:],
                        lhsT=onrm[:, h, stl0],
                        rhs=wo_sb[:, h, esl],
                        start=(h == 0),
                        stop=False,
                    )
                if ec == 0:
                    emit_norm_a2()
            emit_norm_b()
            for ec in range(E // SC):
                esl = slice(ec * SC, (ec + 1) * SC)
                nc.tensor.matmul(
                    pss[ec][:],
                    lhsT=onrm[:, G - 1, stl0],
                    rhs=wo_sb[:, G - 1, esl],
                    start=False,
                    stop=True,
                )
                nc.vector.tensor_copy(out=ob0[:, esl], in_=pss[ec][:])
                nc.gpsimd.dma_start(out=out[stl0, esl], in_=ob0[:, esl])
            for sti in range(1, SC // 128):
                emit_outproj_stripe(NSC - 1, sti)
    nc.compile()
    return nc


_NC_CACHE = None


def _get_nc():
    global _NC_CACHE
    if _NC_CACHE is None:
        _NC_CACHE = build_nc()
    return _NC_CACHE


def _prep_in_maps(query, key, value, attn_mask, Wq, Wk, Wv, Wo):
    query = np.asarray(query, dtype=np.float32)
    key = np.asarray(key, dtype=np.float32)
    value = np.asarray(value, dtype=np.float32)
    Wq = np.asarray(Wq, dtype=np.float32)
    Wk = np.asarray(Wk, dtype=np.float32)
    Wv = np.asarray(Wv, dtype=np.float32)
    Wo = np.asarray(Wo, dtype=np.float32)
    am = np.asarray(attn_mask)

    def xflat(x):
        # [S, E] -> [128, NSC*NET*SC]: row p = for sc, for t:
        # x[sc*SC:(sc+1)*SC, t*128+p]
        a = x.reshape(NSC, SC, NET, 128).transpose(3, 0, 2, 1)
        return np.ascontiguousarray(a.reshape(128, XW)).astype(NPBF)

    def wflat(w_t):  # w_t: [E, M] = W.T; -> [128, NET*M] row p = w_t[t*128+p]
        m = w_t.shape[1]
        a = w_t.reshape(NET, 128, m).transpose(1, 0, 2)
        return np.ascontiguousarray(a.reshape(128, NET * m)).astype(NPBF)

    def wqflat(w_t):  # [E, GD] -> [128, G*NET*D], head-major blocks
        a = w_t.reshape(NET, 128, G, D).transpose(1, 2, 0, 3)
        return np.ascontiguousarray(a.reshape(128, G * NET * D)).astype(NPBF)

    def woflat(w_g):  # w_g: [GD, E]; -> [128, G*E] row p = w_g[h*128+p]
        a = w_g.reshape(G, 128, E).transpose(1, 0, 2)
        return np.ascontiguousarray(a.reshape(128, G * E)).astype(NPBF)

    xqf = [xflat(query[b]) for b in range(B)]
    xkf = [xflat(key[b]) for b in range(B)]
    xvf = [xflat(value[b]) for b in range(B)]

    # [k, q] multiplicative triangle for the diagonal 128x128 blocks,
    # left-padded with 384 zero columns (stale-region zeroing slices)
    m0 = np.asarray(am[0, 0, :128, :128], dtype=np.float32)  # [q, k]
    msk2 = np.zeros((128, 512), np.float32)
    msk2[:, 384:512] = m0.T
    msk2 = np.ascontiguousarray(msk2).astype(NPBF)

    in_maps = []
    for b in range(B):
        for g in range(HKV):
            in_maps.append(
                {
                    "xq": xqf[b],
                    "xk": xkf[b],
                    "xv": xvf[b],
                    "wq": wqflat(
                        np.ascontiguousarray(Wq[g * GD : (g + 1) * GD, :].T)
                    ),
                    "wk": wflat(
                        np.ascontiguousarray(Wk[g * D : (g + 1) * D, :].T)
                    ),
                    "wv": wflat(
                        np.ascontiguousarray(Wv[g * D : (g + 1) * D, :].T)
                    ),
                    "wo": woflat(
                        np.ascontiguousarray(Wo[:, g * GD : (g + 1) * GD].T)
                    ),
                    "msk": msk2,
                }
            )
    return in_maps


def _run(inputs, trace=False, **kw):
    nc = _get_nc()
    in_maps = _prep_in_maps(**inputs)
    res = run_bass_kernel_spmd(
        nc, in_maps, list(range(NCORES)), trace=trace, **kw
    )
    outs = [np.asarray(r["out"]) for r in res.results]
    full = np.empty((B, S, E), dtype=np.float32)
    for b in range(B):
        acc = outs[b * HKV].astype(np.float32)
        for g in range(1, HKV):
            acc = acc + outs[b * HKV + g].astype(np.float32)
        full[b] = acc
    return full, res


def kernel(**inputs):
    full, _ = _run(inputs, trace=False)
    return full
